# revision 1
# baseline (speedup 1.0000x reference)
"""Trainium2 Bass kernel for the signature-kernel (Goursat PDE) problem.

Full inputs: xs (32, 64, 16) f32, ys (32, 64, 16) f32.
Output: (32, 32) f32 signature-kernel Gram matrix.

Strategy (8 NeuronCores, SPMD, no collectives):
  - Shard batch_x across cores: core c owns a in {4c..4c+3} -> 4*32 = 128
    (x, y) pairs, one pair per SBUF partition.
  - Double increments inc[a,b,i,j] = sum_d Dxs[a,i,d] Dys[b,j,d] are computed
    on-device with PE matmuls using a host-built block-diagonal lhsT
    (contraction over (a', d), a'-blocks of Dys) so the output lands directly
    in pair-major partition layout. Inputs ship as bf16 hi/lo splits; each
    product is 3 accumulating bf16 matmuls (hi*hi + hi*lo + lo*hi), giving
    fp32-level accuracy at bf16 PE speed. The producer pipeline (DMA, matmul,
    PSUM copy, coefficient build, dyadic column expansion) is chunked along j
    so the PDE row loop starts after the first chunk.
  - The Goursat PDE recurrence K[i+1,j+1] = c1*(K[i+1,j] + K[i,j+1]) - c2*K[i,j]
    is solved as 126 per-row affine scans x_j = c1_j*x_{j-1} + b_j using the
    DVE TensorTensorScan instruction across all 128 pairs at once (the grid is
    solved transposed - rows=ys-steps - which is valid since the PDE stencil
    is symmetric in (i, j)). Per row, both products c1_j*K[r,j+1] and
    -c2_j*K[r,j] come from ONE [128, 252] tensor_tensor op: the coefficients
    are stored interleaved [c1_j, -c2_j] and the K row is read through a
    double-read access pattern (offset 1+j-s), then b_j is the stride-2
    pair-sum and the scan consumes the even (c1) slots as data0.
"""

import os
import sys

import numpy as np

for _p in ("/opt/trn_rl_repo", "/root/.axon_site", "/root/.axon_site/_ro/trn_rl_repo",
           "/root/.axon_site/_ro/pypackages"):
    if os.path.isdir(_p) and _p not in sys.path:
        sys.path.append(_p)

_STATE: dict = {}

JCH = [(2, 0), (2, 2), (3, 4), (4, 7), (6, 11), (8, 17), (8, 25), (8, 33), (8, 41), (8, 49), (6, 57)]


def _build_program():
    from contextlib import ExitStack

    import concourse.bass as bass
    import concourse.tile as tile
    from concourse import bacc, mybir

    f32 = mybir.dt.float32
    bf16 = mybir.dt.bfloat16
    Alu = mybir.AluOpType
    Act = mybir.ActivationFunctionType

    nc = bacc.Bacc(
        "TRN2",
        target_bir_lowering=False,
        debug=False,
        enable_asserts=True,
        num_devices=8,
    )
    # bd[(a'*16+d), j, (a*32+b)] = delta_{a,a'} * Dys[b, j, d], split hi/lo bf16
    bdh_d = nc.dram_tensor("bdh", [64, 63 * 128], bf16, kind="ExternalInput").ap()
    bdl_d = nc.dram_tensor("bdl", [64, 63 * 128], bf16, kind="ExternalInput").ap()
    dxh_d = nc.dram_tensor("dxh", [64, 63], bf16, kind="ExternalInput").ap()
    dxl_d = nc.dram_tensor("dxl", [64, 63], bf16, kind="ExternalInput").ap()
    out_d = nc.dram_tensor("out", [128, 1], f32, kind="ExternalOutput").ap()

    with ExitStack() as ctx:
        tc = ctx.enter_context(tile.TileContext(nc))
        ws = ctx.enter_context(tc.tile_pool(name="ws", bufs=1))
        pp = ctx.enter_context(tc.tile_pool(name="pp", bufs=1, space="PSUM"))
        ch = ctx.enter_context(tc.tile_pool(name="ch", bufs=2))
        tmp = ctx.enter_context(tc.tile_pool(name="tmp", bufs=2))

        dxh_sb = ws.tile([64, 63], bf16)
        nc.sync.dma_start(out=dxh_sb[:], in_=dxh_d)
        dxl_sb = ws.tile([64, 63], bf16)
        nc.sync.dma_start(out=dxl_sb[:], in_=dxl_d)
        bdh_sb = ws.tile([64, 63, 128], bf16)
        bdl_sb = ws.tile([64, 63, 128], bf16)
        bdh_v = bdh_d.rearrange("k (j p) -> k j p", j=63)
        bdl_v = bdl_d.rearrange("k (j p) -> k j p", j=63)
        for ln, st in JCH:
            nc.sync.dma_start(
                out=bdh_sb[:, st : st + ln, :], in_=bdh_v[:, st : st + ln, :]
            )
            nc.sync.dma_start(
                out=bdl_sb[:, st : st + ln, :], in_=bdl_v[:, st : st + ln, :]
            )

        # Scan-stream K buffers: row K[r, m] lives at slot t = 2m+1 of sc[:, r&1, :]
        # (odd slots of the 252-wide interleaved scan output, shifted by 2);
        # slot 1 is the col-0 boundary (always 1).
        sc = ws.tile([128, 2, 256], f32)
        # K[0, :] = 1 row: readers use odd slots, so filling evens too is fine
        nc.vector.memset(sc[:, 0, :], 1.0)
        nc.vector.memset(sc[:, 1, 1:2], 1.0)

        ps = pp.tile([128, 63, 64], f32)  # strip j at [:, j, 0:63]; 256B stride
        # interleaved full-width coefficient rows: CC[p, h, j, 0] = c1[h-row, j]
        # (column-doubled), CC[p, h, j, 1] = -c2[h-row, j]
        cc = ws.tile([128, 63, 126, 2], f32)
        # scan data0 stream: D0[p, h, j, 0] = c1[h-row, j], D0[p, h, j, 1] = 1.0
        d0 = ws.tile([128, 63, 126, 2], f32)

        for ln, st in JCH:
            jsl = slice(st, st + ln)
            for j in range(st, st + ln):
                # split-precision product: hi*hi + hi*lo + lo*hi (PSUM accum)
                nc.tensor.matmul(
                    ps[:, j, 0:63], bdh_sb[:, j, :], dxh_sb[:], start=True, stop=False
                )
                nc.tensor.matmul(
                    ps[:, j, 0:63], bdh_sb[:, j, :], dxl_sb[:], start=False, stop=False
                )
                nc.tensor.matmul(
                    ps[:, j, 0:63], bdl_sb[:, j, :], dxh_sb[:], start=False, stop=True
                )
            vf = ch.tile([128, ln, 63], f32, tag="vf")
            nc.scalar.copy(vf[:, 0:ln, :], ps[:, jsl, 0:63])
            sq = ch.tile([128, ln, 63], f32, tag="sq")
            nc.vector.tensor_mul(sq[:], vf[:], vf[:])
            m2 = ch.tile([128, ln, 63], f32, tag="m2")  # -c2 = vf^2/12 - 1
            nc.vector.tensor_scalar(
                out=m2[:], in0=sq[:], scalar1=1.0 / 12.0, scalar2=-1.0,
                op0=Alu.mult, op1=Alu.add,
            )
            c1m2 = ch.tile([128, ln, 63], f32, tag="c1m2")  # c1 - 2
            nc.vector.scalar_tensor_tensor(
                c1m2[:], vf[:], 0.5, m2[:], Alu.mult, Alu.add
            )
            # expand columns 2x into the interleaved slots
            c1dup = c1m2[:].unsqueeze(3).broadcast_to((128, ln, 63, 2))
            m2dup = m2[:].unsqueeze(3).broadcast_to((128, ln, 63, 2))
            cc4 = cc[:].rearrange("p h (j t) s -> p h j t s", t=2)
            d04 = d0[:].rearrange("p h (j t) s -> p h j t s", t=2)
            nc.scalar.activation(
                out=cc4[:, jsl, :, :, 0], in_=c1dup, func=Act.Copy,
                bias=2.0, scale=1.0,
            )
            nc.scalar.activation(
                out=cc4[:, jsl, :, :, 1], in_=m2dup, func=Act.Copy,
                bias=0.0, scale=1.0,
            )
            nc.scalar.activation(
                out=d04[:, jsl, :, :, 0], in_=c1dup, func=Act.Copy,
                bias=2.0, scale=1.0,
            )
            nc.scalar.activation(
                out=d04[:, jsl, :, :, 1], in_=c1dup, func=Act.Copy,
                bias=1.0, scale=0.0,
            )

        wt = ws.tile([128, 2, 252], f32)
        for r in range(126):
            h = r >> 1
            pr = r & 1
            nx = 1 - pr
            ccrow2 = cc[:, h, :, :].rearrange("p j s -> p (j s)")  # [128, 252]
            d0row2 = d0[:, h, :, :].rearrange("p j s -> p (j s)")  # [128, 252]
            # K-row double-read: element (j, s) -> K[r, 1+j-s] at slot 3+2j-2s
            base = sc[:, pr, 3:4]
            kpd = bass.AP(
                tensor=base.tensor, offset=base.offset,
                ap=[list(base.ap[0]), [2, 126], [-2, 2]],
            )
            w = wt[:, pr, :]
            nc.vector.tensor_mul(w, ccrow2, kpd)
            # fused scan over the 252-stream: even step s=(c1*s)+W_e, odd
            # step s=(1*s)+W_o -> K[r+1, j+1] lands at output slot 2j+3
            nc.vector.tensor_tensor_scan(
                sc[:, nx, 2:254], d0row2, w, 1.0, Alu.mult, Alu.add
            )

        nc.sync.dma_start(out=out_d, in_=sc[:, 0, 253:254])

    nc.compile()
    return nc


def _get_nc():
    if "nc" not in _STATE:
        _STATE["nc"] = _build_program()
    return _STATE["nc"]


def _make_inputs(xs: np.ndarray, ys: np.ndarray):
    xs = np.asarray(xs, dtype=np.float32)
    ys = np.asarray(ys, dtype=np.float32)
    dxs_all = (xs[:, 1:, :] - xs[:, :-1, :]) * np.float32(0.25)  # (32, 63, 16)
    dys = ys[:, 1:, :] - ys[:, :-1, :]                           # (32, 63, 16)

    dysT = np.ascontiguousarray(dys.transpose(2, 1, 0))          # [d, j, b]
    bd = np.zeros((4, 16, 63, 4, 32), np.float32)
    for g in range(4):
        bd[g, :, :, g, :] = dysT
    bd = np.ascontiguousarray(bd.reshape(64, 63 * 128))

    import ml_dtypes

    bf16 = ml_dtypes.bfloat16
    bdh = bd.astype(bf16)
    bdl = (bd - bdh.astype(np.float32)).astype(bf16)

    in_maps = []
    for c in range(8):
        dxs_c = np.ascontiguousarray(
            dxs_all[4 * c : 4 * c + 4].transpose(0, 2, 1).reshape(64, 63)
        )  # [(a'*16+d), i]
        dxh = dxs_c.astype(bf16)
        dxl = (dxs_c - dxh.astype(np.float32)).astype(bf16)
        in_maps.append({"bdh": bdh, "bdl": bdl, "dxh": dxh, "dxl": dxl})
    return in_maps


def _run(nc, in_maps, **kwargs):
    from concourse.bass_utils import run_bass_kernel_spmd

    return run_bass_kernel_spmd(nc, in_maps, list(range(8)), **kwargs)


def kernel(xs: np.ndarray, ys: np.ndarray) -> np.ndarray:
    nc = _get_nc()
    in_maps = _make_inputs(xs, ys)
    res = _run(nc, in_maps)
    out = np.concatenate(
        [np.asarray(res.results[c]["out"]).reshape(4, 32) for c in range(8)], axis=0
    )
    return out.astype(np.float32)



# revision 3
# speedup vs baseline: 1.0990x; 1.0990x over previous
"""Trainium2 Bass kernel for the signature-kernel (Goursat PDE) problem.

Rescaled-scan formulation: dividing each PDE row by the running product
P_m = prod(c1) of its own row's coefficients turns the first-order
linear column recurrence into a PURE prefix sum:
    t_m = t_{m-1} + alpha_m * t'_m + beta_m * t'_{m-1}
which maps onto ONE custom DVE instruction per row:
    out = scan(ADD, Src0*Src1, init=1)
over a 252-slot interleaved stream (Src0 = precomputed coefficient
stream, Src1 = double-read of the previous row's scan output) at
~1 elem/cycle -- replacing the stock mul + tensor_tensor_scan pair
(503+818 ns) with a single ~420 ns instruction.

Coefficients per coarse plane (h,u):
  within-pair rows (r=2h+1): alpha = c1, beta = -c2/c1          (B-stream)
  transition rows (r=2h):    alpha/beta additionally scaled by the
    cumprod ratio Rc[h,u] = prod_{v<u}(c1[h-1,v]/c1[h,v])^2 and
    G = Rc*rr, computed by a double-slot segmented stock scan
    (reset via d0=0/w=1 slots).                                  (A-stream)
Producer: PE (split-bf16 matmuls -> vf/2 in PSUM), Act (square from
PSUM, d0 build, B-stream interleaves), DVE (coefficient math + the
row loop, which dominates).
"""

import os
import sys

import numpy as np

for _p in ("/opt/trn_rl_repo", "/root/.axon_site", "/root/.axon_site/_ro/trn_rl_repo",
           "/root/.axon_site/_ro/pypackages"):
    if os.path.isdir(_p) and _p not in sys.path:
        sys.path.append(_p)

_STATE: dict = {}

# h-chunks: (len, start), summing to 63
HCH = [(3, 0), (6, 3), (12, 9), (14, 21), (14, 35), (14, 49)]
MAXLN = max(ln for ln, _ in HCH)


def _register_ops():
    import concourse.dve_ops as dve_ops
    from concourse.dve_spec import (
        C0, C1, C2, AluOp, One, Spec, Src0, Src1, scan,
    )
    from concourse.dve_spec import lower as dve_lower
    from concourse.dve_spec import _has_src1
    from concourse.dve_uop import DveOpSpec

    def reg(name, spec, subdim):
        for o in dve_ops.OPS:
            if o.name == name:
                return o
        shas = {}
        for ver in ("v3", "v4"):
            uops = dve_lower(spec, ver=ver)
            shas[ver] = DveOpSpec(
                name=name, opcode=0x11, uops=uops, rd1_en=_has_src1(spec)
            ).sha(ver)
        op = dve_ops.DveOp(name=name, spec=spec, subdim=subdim, uops_sha=shas)
        dve_ops.OPS.append(op)
        dve_ops._SUB_OPCODE_FOR_NAME[name] = (
            dve_ops._CUSTOM_DVE_ROW_BASE + len(dve_ops.OPS) - 1
        )
        dve_ops.CUSTOM_DVE_SPECS[name] = spec
        return op

    # Corrected prefix scan for a fused A+B row pair: all Src1 reads are
    # shifted by the carry (C0 - 1) and the scan seeds from C0, so the
    # outputs inherit the input pollution — the in-flight self-read of
    # the A page by the B page needs the SAME correction, and out[253]
    # (= delta + 1) is exactly the next pair's C0.
    fused = reg(
        "ANT_PDE_PAIR",
        Spec(body=scan(AluOp.ADD, Src0 * (Src1 - (C0 - One)), init=C0)),
        subdim=False,
    )
    # ring-wrap A row: corrected reads, fresh seed (outputs are true)
    wrapa = reg(
        "ANT_PDE_WRAPA",
        Spec(body=scan(AluOp.ADD, Src0 * (Src1 - (C0 - One)), init=One)),
        subdim=False,
    )
    # ring-wrap B row: uncorrected reads (of the true A outputs), seeded
    # from the A-row final so its outputs follow the pollution convention
    wrapb = reg(
        "ANT_PDE_WRAPB",
        Spec(body=scan(AluOp.ADD, Src0 * Src1, init=C0)),
        subdim=False,
    )
    # bB = (c2m - 2) * r1 in one 1-elem/cycle op
    bbop = reg(
        "ANT_ADDC_MUL",
        Spec(body=(Src0 + C2) * Src1),
        subdim=False,
    )
    return fused, wrapa, wrapb, bbop


def _build_program():
    from contextlib import ExitStack

    import concourse.bass as bass
    import concourse.tile as tile
    from concourse import bacc, mybir

    FUSED, WRAPA, WRAPB, BBOP = _register_ops()
    R = 15  # ring pair-slots; wraps at h % R == 0 (h > 0)

    f32 = mybir.dt.float32
    bf16 = mybir.dt.bfloat16
    Alu = mybir.AluOpType
    Act = mybir.ActivationFunctionType

    nc = bacc.Bacc(
        "TRN2",
        target_bir_lowering=False,
        debug=False,
        enable_asserts=True,
        num_devices=8,
    )
    bdh_d = nc.dram_tensor("bdh", [64, 63 * 128], bf16, kind="ExternalInput").ap()
    bdl_d = nc.dram_tensor("bdl", [64, 63 * 128], bf16, kind="ExternalInput").ap()
    dxh_d = nc.dram_tensor("dxh", [64, 63], bf16, kind="ExternalInput").ap()
    dxl_d = nc.dram_tensor("dxl", [64, 63], bf16, kind="ExternalInput").ap()
    out_d = nc.dram_tensor("out", [128, 1], f32, kind="ExternalOutput").ap()

    with ExitStack() as ctx:
        tc = ctx.enter_context(tile.TileContext(nc))
        ws = ctx.enter_context(tc.tile_pool(name="ws", bufs=1))
        pp = ctx.enter_context(tc.tile_pool(name="pp", bufs=2, space="PSUM"))
        ch = ctx.enter_context(tc.tile_pool(name="ch", bufs=2))
        bd = ctx.enter_context(tc.tile_pool(name="bd", bufs=2))
        cf = ctx.enter_context(tc.tile_pool(name="cf", bufs=2))

        dxh_sb = ws.tile([64, 63], bf16)
        nc.sync.dma_start(out=dxh_sb[:], in_=dxh_d)
        dxl_sb = ws.tile([64, 63], bf16)
        nc.sync.dma_start(out=dxl_sb[:], in_=dxl_d)
        bdh_v = bdh_d.rearrange("k (j p) -> k j p", j=63)
        bdl_v = bdl_d.rearrange("k (j p) -> k j p", j=63)

        # persistent c1 plane with a leading all-ones h-row (for rr's h-1 read)
        c1x = ws.tile([128, 64, 63], f32)
        nc.gpsimd.memset(c1x[:, 0, :], 1.0)
        # t-row ring: slot 0 = all-ones pseudo-pair, pair h -> slot 1+(h%R);
        # each slot = [A page (254) | B page (254)]
        ring = ws.tile([128, R + 1, 508], f32)
        nc.gpsimd.memset(ring[:, 0, :], 1.0)
        # segmented-cumprod d0 buffers: col 0 = reset (0), cols 1..126 = rr
        # doubled, col 127 = 0 pad
        d0a = ws.tile([128, MAXLN, 128], f32)
        d0b = ws.tile([128, MAXLN, 128], f32)
        nc.gpsimd.memset(d0a[:, :, 0:1], 0.0)
        nc.gpsimd.memset(d0a[:, :, 127:128], 0.0)
        nc.gpsimd.memset(d0b[:, :, 0:1], 0.0)
        nc.gpsimd.memset(d0b[:, :, 127:128], 0.0)
        # one-hot w planes for the scan (1 at each reset slot, else 0)
        wone = ws.tile([128, MAXLN, 128], f32)
        nc.gpsimd.memset(wone[:], 0.0)
        nc.gpsimd.memset(wone[:, :, 0:1], 1.0)

        for ci, (ln, st) in enumerate(HCH):
            hs = slice(st, st + ln)
            bdh_sb = bd.tile([64, ln, 128], bf16, tag="bdh")
            nc.sync.dma_start(out=bdh_sb[:], in_=bdh_v[:, hs, :])
            bdl_sb = bd.tile([64, ln, 128], bf16, tag="bdl")
            nc.sync.dma_start(out=bdl_sb[:], in_=bdl_v[:, hs, :])

            # ps = vf/2 (dx carries 0.125 = 0.25 inc-scale * 0.5)
            ps = pp.tile([128, ln, 64], f32, tag="ps")
            for j in range(ln):
                nc.tensor.matmul(
                    ps[:, j, 0:63], bdh_sb[:, j, :], dxh_sb[:], start=True, stop=False
                )
                nc.tensor.matmul(
                    ps[:, j, 0:63], bdh_sb[:, j, :], dxl_sb[:], start=False, stop=False
                )
                nc.tensor.matmul(
                    ps[:, j, 0:63], bdl_sb[:, j, :], dxh_sb[:], start=False, stop=True
                )

            # sq = vf^2 = (2 * ps)^2  (Act reads PSUM)
            sq = ch.tile([128, ln, 63], f32, tag="sq")
            nc.scalar.activation(out=sq[:], in_=ps[:, :, 0:63], func=Act.Square,
                                 bias=0.0, scale=2.0)
            # c2m = 1 + vf^2/12   (m2 = -c2 = c2m - 2)
            c2m = ch.tile([128, ln, 63], f32, tag="c2m")
            nc.vector.tensor_scalar(
                out=c2m[:], in0=sq[:], scalar1=1.0 / 12.0, scalar2=1.0,
                op0=Alu.mult, op1=Alu.add,
            )
            # c1 = c2m + vf/2
            c1c = c1x[:, 1 + st : 1 + st + ln, :]
            nc.vector.tensor_add(c1c, c2m[:], ps[:, :, 0:63])
            # r1 = 1/c1 (~2 ULP, two custom ops)
            r1 = ch.tile([128, ln, 63], f32, tag="r1")
            scr = ch.tile([128, ln, 63], f32, tag="scr")
            nc.vector.reciprocal_approx_accurate(
                out=r1[:].rearrange("p h u -> p (h u)"),
                in_=c1c.rearrange("p h u -> p (h u)"),
                scratch=scr[:].rearrange("p h u -> p (h u)"),
            )
            # bB = -c2/c1 = (c2m - 2) * r1
            bB = ch.tile([128, ln, 63], f32, tag="bB")
            nc.vector._custom_dve(
                BBOP,
                out=bB[:].rearrange("p h u -> p (h u)"),
                in0=c2m[:].rearrange("p h u -> p (h u)"),
                in1=r1[:].rearrange("p h u -> p (h u)"),
                imm2=-2.0,
            )
            # rr = c1[h-1]/c1[h]
            rr = ch.tile([128, ln, 63], f32, tag="rr")
            nc.vector.tensor_mul(rr[:], c1x[:, st : st + ln, :], r1[:])
            # d0 cols 1..126 = rr doubled (Act); cols 0/127 stay 0
            d0 = d0a if ci % 2 == 0 else d0b
            rrdup = rr[:].unsqueeze(3).broadcast_to((128, ln, 63, 2))
            d0v = d0[:, 0:ln, 1:127].rearrange("p h (u t) -> p h u t", t=2)
            nc.scalar.activation(out=d0v, in_=rrdup, func=Act.Copy,
                                 bias=0.0, scale=1.0)
            # segmented double-slot cumprod: per h, slot 0 resets state to 1,
            # then pairs (x rr -> G[h,u], x rr -> Rc[h,u+1])
            rg = ch.tile([128, ln, 128], f32, tag="rg")
            nc.vector.tensor_tensor_scan(
                rg[:].rearrange("p h u -> p (h u)"),
                d0[:, 0:ln, :].rearrange("p h u -> p (h u)"),
                wone[:, 0:ln, :].rearrange("p h u -> p (h u)"),
                1.0, Alu.mult, Alu.add,
            )

            # coefficient streams: cofc[:, rl, 252]; A-rows at even rl,
            # B-rows at odd rl. Slot layout per u: [alpha_e, beta_e,
            # alpha_o, beta_o] = A: [c1*G, bB*Rc, c1*Rc+, bB*G],
            # B: [c1, bB, c1, bB].
            # per-pair streams: [page(2), 254] with 2-slot zero boundary pairs
            cofc = cf.tile([128, ln, 2, 254], f32, tag="cof")
            nc.gpsimd.memset(cofc[:, :, :, 0:2], 0.0)
            c1dup = c1c.unsqueeze(3).broadcast_to((128, ln, 63, 2))
            bBdup = bB[:].unsqueeze(3).broadcast_to((128, ln, 63, 2))
            # coefficient region slots 2+4u+2w+e viewed as (u, w, e)
            cv = cofc[:, :, :, 2:254].rearrange(
                "p i par (u w e) -> p i par u w e", w=2, e=2
            )
            # [c1*G, c1*Rc+] -> A slots (4u, 4u+2)
            nc.vector.tensor_mul(
                cv[:, :, 0, :, :, 0],
                c1dup,
                rg[:, :, 1:127].rearrange("p h (u t) -> p h u t", t=2),
            )
            # [bB*Rc, bB*G] -> A slots (4u+1, 4u+3)
            nc.vector.tensor_mul(
                cv[:, :, 0, :, :, 1],
                bBdup,
                rg[:, :, 0:126].rearrange("p h (u t) -> p h u t", t=2),
            )
            nc.scalar.activation(out=cv[:, :, 1, :, :, 0], in_=c1dup,
                                 func=Act.Copy, bias=0.0, scale=1.0)
            nc.scalar.activation(out=cv[:, :, 1, :, :, 1], in_=bBdup,
                                 func=Act.Copy, bias=0.0, scale=1.0)

            # the row loop: one fused op per pair (two CORR ops at ring wraps)
            for il in range(ln):
                h = st + il
                s = 1 + (h % R)
                p = 0 if h == 0 else 1 + ((h - 1) % R)
                prev = ring[:, p, 255:256]
                if h > 0 and h % R == 0:
                    # wrap: A row (reads prev slot's B page at ring end)
                    pA = bass.AP(
                        tensor=prev.tensor, offset=prev.offset,
                        ap=[list(prev.ap[0]), [2, 127], [-2, 2]],
                    )
                    nc.vector._custom_dve(
                        WRAPA,
                        out=ring[:, s, 0:254],
                        in0=cofc[:, il, 0, :],
                        in1=pA,
                        s0=ring[:, p, 253:254],
                    )
                    own = ring[:, s, 1:2]
                    pB = bass.AP(
                        tensor=own.tensor, offset=own.offset,
                        ap=[list(own.ap[0]), [2, 127], [-2, 2]],
                    )
                    nc.vector._custom_dve(
                        WRAPB,
                        out=ring[:, s, 254:508],
                        in0=cofc[:, il, 1, :],
                        in1=pB,
                        s0=ring[:, s, 253:254],
                    )
                else:
                    kpd = bass.AP(
                        tensor=prev.tensor, offset=prev.offset,
                        ap=[list(prev.ap[0]), [2, 254], [-2, 2]],
                    )
                    nc.vector._custom_dve(
                        FUSED,
                        out=ring[:, s, :],
                        in0=cofc[:, il, :, :].rearrange("p par k -> p (par k)"),
                        in1=kpd,
                        s0=ring[:, p, 253:254],
                    )

        # final: true t = (B-final - A-final + 1); K = t * (prod_u c1[62,u])^2
        send = 1 + (62 % R)
        tb = ws.tile([128, 1], f32)
        nc.vector.tensor_tensor(
            tb[:], ring[:, send, 507:508], ring[:, send, 253:254], Alu.subtract
        )
        tcp = ws.tile([128, 64], f32)
        nc.vector.tensor_tensor_scan(
            tcp[:, 0:63], c1x[:, 63, :], wone[:, 0, 1:64], 1.0, Alu.mult, Alu.add
        )
        fin = ws.tile([128, 1], f32)
        nc.vector.tensor_mul(fin[:], tcp[:, 62:63], tcp[:, 62:63])
        res = ws.tile([128, 1], f32)
        nc.vector.scalar_tensor_tensor(
            res[:], tb[:], 1.0, fin[:], Alu.add, Alu.mult
        )
        nc.sync.dma_start(out=out_d, in_=res[:])

    nc.compile()
    return nc


def _get_nc():
    if "nc" not in _STATE:
        _STATE["nc"] = _build_program()
    return _STATE["nc"]


def _make_inputs(xs: np.ndarray, ys: np.ndarray):
    xs = np.asarray(xs, dtype=np.float32)
    ys = np.asarray(ys, dtype=np.float32)
    # 0.125 = 0.25 (inc scale 1/d^2) * 0.5 (so PSUM holds vf/2)
    dxs_all = (xs[:, 1:, :] - xs[:, :-1, :]) * np.float32(0.125)  # (32, 63, 16)
    dys = ys[:, 1:, :] - ys[:, :-1, :]                            # (32, 63, 16)

    dysT = np.ascontiguousarray(dys.transpose(2, 1, 0))           # [d, j, b]
    bd = np.zeros((4, 16, 63, 4, 32), np.float32)
    for g in range(4):
        bd[g, :, :, g, :] = dysT
    bd = np.ascontiguousarray(bd.reshape(64, 63 * 128))

    import ml_dtypes

    bf16 = ml_dtypes.bfloat16
    bdh = bd.astype(bf16)
    bdl = (bd - bdh.astype(np.float32)).astype(bf16)

    in_maps = []
    for c in range(8):
        dxs_c = np.ascontiguousarray(
            dxs_all[4 * c : 4 * c + 4].transpose(0, 2, 1).reshape(64, 63)
        )  # [(a'*16+d), i]
        dxh = dxs_c.astype(bf16)
        dxl = (dxs_c - dxh.astype(np.float32)).astype(bf16)
        in_maps.append({"bdh": bdh, "bdl": bdl, "dxh": dxh, "dxl": dxl})
    return in_maps


def _run(nc, in_maps, **kwargs):
    from concourse.bass_utils import run_bass_kernel_spmd

    return run_bass_kernel_spmd(nc, in_maps, list(range(8)), **kwargs)


def kernel(xs: np.ndarray, ys: np.ndarray) -> np.ndarray:
    nc = _get_nc()
    in_maps = _make_inputs(xs, ys)
    res = _run(nc, in_maps)
    out = np.concatenate(
        [np.asarray(res.results[c]["out"]).reshape(4, 32) for c in range(8)], axis=0
    )
    return out.astype(np.float32)


# revision 4
# speedup vs baseline: 1.1194x; 1.0186x over previous
"""Trainium2 Bass kernel for the signature-kernel (Goursat PDE) problem.

Rescaled-scan formulation: dividing each PDE row by the running product
P_m = prod(c1) of its own row's coefficients turns the first-order
linear column recurrence into a PURE prefix sum:
    t_m = t_{m-1} + alpha_m * t'_m + beta_m * t'_{m-1}
which maps onto ONE custom DVE instruction per row:
    out = scan(ADD, Src0*Src1, init=1)
over a 252-slot interleaved stream (Src0 = precomputed coefficient
stream, Src1 = double-read of the previous row's scan output) at
~1 elem/cycle -- replacing the stock mul + tensor_tensor_scan pair
(503+818 ns) with a single ~420 ns instruction.

Coefficients per coarse plane (h,u):
  within-pair rows (r=2h+1): alpha = c1, beta = -c2/c1          (B-stream)
  transition rows (r=2h):    alpha/beta additionally scaled by the
    cumprod ratio Rc[h,u] = prod_{v<u}(c1[h-1,v]/c1[h,v])^2 and
    G = Rc*rr, computed by a double-slot segmented stock scan
    (reset via d0=0/w=1 slots).                                  (A-stream)
Producer: PE (split-bf16 matmuls -> vf/2 in PSUM), Act (square from
PSUM, d0 build, B-stream interleaves), DVE (coefficient math + the
row loop, which dominates).
"""

import os
import sys

import numpy as np

for _p in ("/opt/trn_rl_repo", "/root/.axon_site", "/root/.axon_site/_ro/trn_rl_repo",
           "/root/.axon_site/_ro/pypackages"):
    if os.path.isdir(_p) and _p not in sys.path:
        sys.path.append(_p)

_STATE: dict = {}

# h-chunks: (len, start), summing to 63
HCH = [(3, 0), (6, 3), (12, 9), (14, 21), (14, 35), (14, 49)]
MAXLN = max(ln for ln, _ in HCH)


def _register_ops():
    import concourse.dve_ops as dve_ops
    from concourse.dve_spec import (
        C0, C1, C2, AluOp, One, Spec, Src0, Src1, scan, sq,
    )
    from concourse.dve_spec import lower as dve_lower
    from concourse.dve_spec import _has_src1
    from concourse.dve_uop import DveOpSpec

    def reg(name, spec, subdim):
        for o in dve_ops.OPS:
            if o.name == name:
                return o
        shas = {}
        for ver in ("v3", "v4"):
            uops = dve_lower(spec, ver=ver)
            shas[ver] = DveOpSpec(
                name=name, opcode=0x11, uops=uops, rd1_en=_has_src1(spec)
            ).sha(ver)
        op = dve_ops.DveOp(name=name, spec=spec, subdim=subdim, uops_sha=shas)
        dve_ops.OPS.append(op)
        dve_ops._SUB_OPCODE_FOR_NAME[name] = (
            dve_ops._CUSTOM_DVE_ROW_BASE + len(dve_ops.OPS) - 1
        )
        dve_ops.CUSTOM_DVE_SPECS[name] = spec
        return op

    # Corrected prefix scan for a fused A+B row pair: all Src1 reads are
    # shifted by the carry (C0 - 1) and the scan seeds from C0, so the
    # outputs inherit the input pollution — the in-flight self-read of
    # the A page by the B page needs the SAME correction, and out[253]
    # (= delta + 1) is exactly the next pair's C0.
    fused = reg(
        "ANT_PDE_PAIR",
        Spec(body=scan(AluOp.ADD, Src0 * (Src1 - (C0 - One)), init=C0)),
        subdim=False,
    )
    # ring-wrap A row: corrected reads, fresh seed (outputs are true)
    wrapa = reg(
        "ANT_PDE_WRAPA",
        Spec(body=scan(AluOp.ADD, Src0 * (Src1 - (C0 - One)), init=One)),
        subdim=False,
    )
    # ring-wrap B row: uncorrected reads (of the true A outputs), seeded
    # from the A-row final so its outputs follow the pollution convention
    wrapb = reg(
        "ANT_PDE_WRAPB",
        Spec(body=scan(AluOp.ADD, Src0 * Src1, init=C0)),
        subdim=False,
    )
    # ps = vf/2 in PSUM; w = c1 - 1 = ps + ps^2/3 (C2 = 1/3)
    _w = Src0 + sq(Src0) * C2
    # c1 = 1 + w
    c1op = reg("ANT_C1_FROM_PS", Spec(body=One + _w), subdim=False)
    # r1 = 1/(1+w) ~ 1 - w(1 - w(1 - w)); |w| <~ 0.05 -> err ~ w^4
    r1op = reg(
        "ANT_RECIP_SERIES",
        Spec(body=One - _w * (One - _w * (One - _w))),
        subdim=False,
    )
    # bB = -c2/c1 = (ps^2/3 - 1) * r1
    bbop = reg(
        "ANT_BB_FROM_PS",
        Spec(body=(sq(Src0) * C2 - One) * Src1),
        subdim=False,
    )
    return fused, wrapa, wrapb, c1op, r1op, bbop


def _build_program():
    from contextlib import ExitStack

    import concourse.bass as bass
    import concourse.tile as tile
    from concourse import bacc, mybir

    FUSED, WRAPA, WRAPB, C1OP, R1OP, BBOP = _register_ops()
    R = 15  # ring pair-slots; wraps at h % R == 0 (h > 0)

    f32 = mybir.dt.float32
    bf16 = mybir.dt.bfloat16
    Alu = mybir.AluOpType
    Act = mybir.ActivationFunctionType

    nc = bacc.Bacc(
        "TRN2",
        target_bir_lowering=False,
        debug=False,
        enable_asserts=True,
        num_devices=8,
    )
    bdh_d = nc.dram_tensor("bdh", [64, 63 * 128], bf16, kind="ExternalInput").ap()
    bdl_d = nc.dram_tensor("bdl", [64, 63 * 128], bf16, kind="ExternalInput").ap()
    dxh_d = nc.dram_tensor("dxh", [64, 63], bf16, kind="ExternalInput").ap()
    dxl_d = nc.dram_tensor("dxl", [64, 63], bf16, kind="ExternalInput").ap()
    out_d = nc.dram_tensor("out", [128, 1], f32, kind="ExternalOutput").ap()

    with ExitStack() as ctx:
        tc = ctx.enter_context(tile.TileContext(nc))
        ws = ctx.enter_context(tc.tile_pool(name="ws", bufs=1))
        pp = ctx.enter_context(tc.tile_pool(name="pp", bufs=2, space="PSUM"))
        ch = ctx.enter_context(tc.tile_pool(name="ch", bufs=2))
        bd = ctx.enter_context(tc.tile_pool(name="bd", bufs=2))
        cf = ctx.enter_context(tc.tile_pool(name="cf", bufs=2))

        dxh_sb = ws.tile([64, 63], bf16)
        nc.sync.dma_start(out=dxh_sb[:], in_=dxh_d)
        dxl_sb = ws.tile([64, 63], bf16)
        nc.sync.dma_start(out=dxl_sb[:], in_=dxl_d)
        bdh_v = bdh_d.rearrange("k (j p) -> k j p", j=63)
        bdl_v = bdl_d.rearrange("k (j p) -> k j p", j=63)

        # persistent c1 plane with a leading all-ones h-row (for rr's h-1 read)
        c1x = ws.tile([128, 64, 63], f32)
        nc.gpsimd.memset(c1x[:, 0, :], 1.0)
        # t-row ring: slot 0 = all-ones pseudo-pair, pair h -> slot 1+(h%R);
        # each slot = [A page (254) | B page (254)]
        ring = ws.tile([128, R + 1, 508], f32)
        nc.gpsimd.memset(ring[:, 0, :], 1.0)
        # segmented-cumprod d0 buffers: col 0 = reset (0), cols 1..126 = rr
        # doubled, col 127 = 0 pad
        d0a = ws.tile([128, MAXLN, 128], f32)
        d0b = ws.tile([128, MAXLN, 128], f32)
        nc.gpsimd.memset(d0a[:, :, 0:1], 0.0)
        nc.gpsimd.memset(d0a[:, :, 127:128], 0.0)
        nc.gpsimd.memset(d0b[:, :, 0:1], 0.0)
        nc.gpsimd.memset(d0b[:, :, 127:128], 0.0)
        # one-hot w planes for the scan (1 at each reset slot, else 0)
        wone = ws.tile([128, MAXLN, 128], f32)
        nc.gpsimd.memset(wone[:], 0.0)
        nc.gpsimd.memset(wone[:, :, 0:1], 1.0)

        for ci, (ln, st) in enumerate(HCH):
            hs = slice(st, st + ln)
            bdh_sb = bd.tile([64, ln, 128], bf16, tag="bdh")
            nc.sync.dma_start(out=bdh_sb[:], in_=bdh_v[:, hs, :])
            bdl_sb = bd.tile([64, ln, 128], bf16, tag="bdl")
            nc.sync.dma_start(out=bdl_sb[:], in_=bdl_v[:, hs, :])

            # ps = vf/2 (dx carries 0.125 = 0.25 inc-scale * 0.5)
            ps = pp.tile([128, ln, 64], f32, tag="ps")
            for j in range(ln):
                nc.tensor.matmul(
                    ps[:, j, 0:63], bdh_sb[:, j, :], dxh_sb[:], start=True, stop=False
                )
                nc.tensor.matmul(
                    ps[:, j, 0:63], bdh_sb[:, j, :], dxl_sb[:], start=False, stop=False
                )
                nc.tensor.matmul(
                    ps[:, j, 0:63], bdl_sb[:, j, :], dxh_sb[:], start=False, stop=True
                )

            # c1 = 1 + ps + ps^2/3, straight from PSUM
            c1c = c1x[:, 1 + st : 1 + st + ln, :]
            nc.vector._custom_dve(
                C1OP, out=c1c, in0=ps[:, :, 0:63], imm2=1.0 / 3.0,
            )
            # r1 = 1/c1 via 3-term Horner series (err ~ w^4 < 1e-5)
            r1 = ch.tile([128, ln, 63], f32, tag="r1")
            nc.vector._custom_dve(
                R1OP, out=r1[:], in0=ps[:, :, 0:63], imm2=1.0 / 3.0,
            )
            # bB = -c2/c1 = (ps^2/3 - 1) * r1
            bB = ch.tile([128, ln, 63], f32, tag="bB")
            nc.vector._custom_dve(
                BBOP,
                out=bB[:],
                in0=ps[:, :, 0:63],
                in1=r1[:].rearrange("p h u -> p (h u)"),
                imm2=1.0 / 3.0,
            )
            # rr = c1[h-1]/c1[h]
            rr = ch.tile([128, ln, 63], f32, tag="rr")
            nc.vector.tensor_mul(rr[:], c1x[:, st : st + ln, :], r1[:])
            # d0 cols 1..126 = rr doubled (Act); cols 0/127 stay 0
            d0 = d0a if ci % 2 == 0 else d0b
            rrdup = rr[:].unsqueeze(3).broadcast_to((128, ln, 63, 2))
            d0v = d0[:, 0:ln, 1:127].rearrange("p h (u t) -> p h u t", t=2)
            nc.scalar.activation(out=d0v, in_=rrdup, func=Act.Copy,
                                 bias=0.0, scale=1.0)
            # segmented double-slot cumprod: per h, slot 0 resets state to 1,
            # then pairs (x rr -> G[h,u], x rr -> Rc[h,u+1])
            rg = ch.tile([128, ln, 128], f32, tag="rg")
            nc.vector.tensor_tensor_scan(
                rg[:].rearrange("p h u -> p (h u)"),
                d0[:, 0:ln, :].rearrange("p h u -> p (h u)"),
                wone[:, 0:ln, :].rearrange("p h u -> p (h u)"),
                1.0, Alu.mult, Alu.add,
            )

            # coefficient streams: cofc[:, rl, 252]; A-rows at even rl,
            # B-rows at odd rl. Slot layout per u: [alpha_e, beta_e,
            # alpha_o, beta_o] = A: [c1*G, bB*Rc, c1*Rc+, bB*G],
            # B: [c1, bB, c1, bB].
            # per-pair streams: [page(2), 254] with 2-slot zero boundary pairs
            cofc = cf.tile([128, ln, 2, 254], f32, tag="cof")
            nc.gpsimd.memset(cofc[:, :, :, 0:2], 0.0)
            c1dup = c1c.unsqueeze(3).broadcast_to((128, ln, 63, 2))
            bBdup = bB[:].unsqueeze(3).broadcast_to((128, ln, 63, 2))
            # coefficient region slots 2+4u+2w+e viewed as (u, w, e)
            cv = cofc[:, :, :, 2:254].rearrange(
                "p i par (u w e) -> p i par u w e", w=2, e=2
            )
            # [c1*G, c1*Rc+] -> A slots (4u, 4u+2)
            nc.vector.tensor_mul(
                cv[:, :, 0, :, :, 0],
                c1dup,
                rg[:, :, 1:127].rearrange("p h (u t) -> p h u t", t=2),
            )
            # [bB*Rc, bB*G] -> A slots (4u+1, 4u+3)
            nc.vector.tensor_mul(
                cv[:, :, 0, :, :, 1],
                bBdup,
                rg[:, :, 0:126].rearrange("p h (u t) -> p h u t", t=2),
            )
            nc.scalar.activation(out=cv[:, :, 1, :, :, 0], in_=c1dup,
                                 func=Act.Copy, bias=0.0, scale=1.0)
            nc.scalar.activation(out=cv[:, :, 1, :, :, 1], in_=bBdup,
                                 func=Act.Copy, bias=0.0, scale=1.0)

            # the row loop: one fused op per pair (two CORR ops at ring wraps)
            for il in range(ln):
                h = st + il
                s = 1 + (h % R)
                p = 0 if h == 0 else 1 + ((h - 1) % R)
                prev = ring[:, p, 255:256]
                if h > 0 and h % R == 0:
                    # wrap: A row (reads prev slot's B page at ring end)
                    pA = bass.AP(
                        tensor=prev.tensor, offset=prev.offset,
                        ap=[list(prev.ap[0]), [2, 127], [-2, 2]],
                    )
                    nc.vector._custom_dve(
                        WRAPA,
                        out=ring[:, s, 0:254],
                        in0=cofc[:, il, 0, :],
                        in1=pA,
                        s0=ring[:, p, 253:254],
                    )
                    own = ring[:, s, 1:2]
                    pB = bass.AP(
                        tensor=own.tensor, offset=own.offset,
                        ap=[list(own.ap[0]), [2, 127], [-2, 2]],
                    )
                    nc.vector._custom_dve(
                        WRAPB,
                        out=ring[:, s, 254:508],
                        in0=cofc[:, il, 1, :],
                        in1=pB,
                        s0=ring[:, s, 253:254],
                    )
                else:
                    kpd = bass.AP(
                        tensor=prev.tensor, offset=prev.offset,
                        ap=[list(prev.ap[0]), [2, 254], [-2, 2]],
                    )
                    nc.vector._custom_dve(
                        FUSED,
                        out=ring[:, s, :],
                        in0=cofc[:, il, :, :].rearrange("p par k -> p (par k)"),
                        in1=kpd,
                        s0=ring[:, p, 253:254],
                    )

        # final: true t = (B-final - A-final + 1); K = t * (prod_u c1[62,u])^2
        send = 1 + (62 % R)
        tb = ws.tile([128, 1], f32)
        nc.vector.tensor_tensor(
            tb[:], ring[:, send, 507:508], ring[:, send, 253:254], Alu.subtract
        )
        tcp = ws.tile([128, 64], f32)
        nc.vector.tensor_tensor_scan(
            tcp[:, 0:63], c1x[:, 63, :], wone[:, 0, 1:64], 1.0, Alu.mult, Alu.add
        )
        fin = ws.tile([128, 1], f32)
        nc.vector.tensor_mul(fin[:], tcp[:, 62:63], tcp[:, 62:63])
        res = ws.tile([128, 1], f32)
        nc.vector.scalar_tensor_tensor(
            res[:], tb[:], 1.0, fin[:], Alu.add, Alu.mult
        )
        nc.sync.dma_start(out=out_d, in_=res[:])

    nc.compile()
    return nc


def _get_nc():
    if "nc" not in _STATE:
        _STATE["nc"] = _build_program()
    return _STATE["nc"]


def _make_inputs(xs: np.ndarray, ys: np.ndarray):
    xs = np.asarray(xs, dtype=np.float32)
    ys = np.asarray(ys, dtype=np.float32)
    # 0.125 = 0.25 (inc scale 1/d^2) * 0.5 (so PSUM holds vf/2)
    dxs_all = (xs[:, 1:, :] - xs[:, :-1, :]) * np.float32(0.125)  # (32, 63, 16)
    dys = ys[:, 1:, :] - ys[:, :-1, :]                            # (32, 63, 16)

    dysT = np.ascontiguousarray(dys.transpose(2, 1, 0))           # [d, j, b]
    bd = np.zeros((4, 16, 63, 4, 32), np.float32)
    for g in range(4):
        bd[g, :, :, g, :] = dysT
    bd = np.ascontiguousarray(bd.reshape(64, 63 * 128))

    import ml_dtypes

    bf16 = ml_dtypes.bfloat16
    bdh = bd.astype(bf16)
    bdl = (bd - bdh.astype(np.float32)).astype(bf16)

    in_maps = []
    for c in range(8):
        dxs_c = np.ascontiguousarray(
            dxs_all[4 * c : 4 * c + 4].transpose(0, 2, 1).reshape(64, 63)
        )  # [(a'*16+d), i]
        dxh = dxs_c.astype(bf16)
        dxl = (dxs_c - dxh.astype(np.float32)).astype(bf16)
        in_maps.append({"bdh": bdh, "bdl": bdl, "dxh": dxh, "dxl": dxl})
    return in_maps


def _run(nc, in_maps, **kwargs):
    from concourse.bass_utils import run_bass_kernel_spmd

    return run_bass_kernel_spmd(nc, in_maps, list(range(8)), **kwargs)


def kernel(xs: np.ndarray, ys: np.ndarray) -> np.ndarray:
    nc = _get_nc()
    in_maps = _make_inputs(xs, ys)
    res = _run(nc, in_maps)
    out = np.concatenate(
        [np.asarray(res.results[c]["out"]).reshape(4, 32) for c in range(8)], axis=0
    )
    return out.astype(np.float32)
